# revision 8
# baseline (speedup 1.0000x reference)
"""Differential self-attention on 8 Trainium2 NeuronCores.

Sharding: batch x head-group. Core c handles batch b = c//4 and heads
hs = 4*(c%4) .. 4*(c%4)+4 (4 of 16 heads). Each core computes q/k/v
projections for its heads, RoPE, both causal softmax score matrices
(flash-style, unnormalized, with matmul-computed row sums), the
differential combination + RMS norm, and a partial out-projection over
its heads' dims. Host sums the 4 partial y per batch and adds bo.

Per-core kernel layouts (feature-major "T" = [feat, seq]):
  qT/kT  [128, 4, 2048]  rows 0:64 = component 1, 64:128 = component 2
  v      [128, 16, 4, 65] (key-block, head, hd + ones col [unused])
  scores sT [j=128, i<=512] so attn@v contracts j on partitions.
"""

import math

import numpy as np
import ml_dtypes

B, S, D = 2, 2048, 1024
H, HD = 16, 64
HALF = HD // 2
NCORES = 8
NH = 4            # heads per core
LAMBDA_INIT = 0.2
EPS = 1e-6
CH = 512          # query chunk
NCH = S // CH
JB = 128          # key block
BF16 = ml_dtypes.bfloat16

_cache = {}


def _build():
    import concourse.bass as bass
    import concourse.tile as tile
    from concourse import bacc, mybir

    F32R = mybir.dt.float32r
    F32 = mybir.dt.float32
    BF = mybir.dt.bfloat16
    AF = mybir.ActivationFunctionType

    nc = bacc.Bacc("TRN2", debug=False, num_devices=NCORES)

    xT = nc.dram_tensor("xT", [D, S], F32R, kind="ExternalInput")
    wqT = nc.dram_tensor("wqT", [128, 8, 512], F32R, kind="ExternalInput")
    wkT = nc.dram_tensor("wkT", [128, 8, 512], F32R, kind="ExternalInput")
    wvT = nc.dram_tensor("wvT", [128, 8, 256], F32R, kind="ExternalInput")
    bqk = nc.dram_tensor("bqk", [8, 128], F32R, kind="ExternalInput")
    bv = nc.dram_tensor("bv", [1, 256], F32R, kind="ExternalInput")
    woT = nc.dram_tensor("woT", [128, 2, 1024], BF, kind="ExternalInput")
    ropeA = nc.dram_tensor("ropeA", [128, S], BF, kind="ExternalInput")
    ropeB = nc.dram_tensor("ropeB", [128, S], BF, kind="ExternalInput")
    maskd = nc.dram_tensor("maskd", [128, 128], BF, kind="ExternalInput")
    lamv = nc.dram_tensor("lamv", [128, 1], F32, kind="ExternalInput")
    iim = nc.dram_tensor("iim", [128, 64], BF, kind="ExternalInput")
    onesd = nc.dram_tensor("onesd", [1, 512], F32R, kind="ExternalInput")
    y_out = nc.dram_tensor("y", [S, D], F32, kind="ExternalOutput")

    xT_r = xT.ap().rearrange("(t p) s -> p t s", p=128)

    with tile.TileContext(nc) as tc:
        import contextlib
        ctx = contextlib.ExitStack()
        with ctx:
            persist = ctx.enter_context(tc.tile_pool(name="persist", bufs=1))
            xpool = ctx.enter_context(tc.tile_pool(name="xc", bufs=2))
            rpool = ctx.enter_context(tc.tile_pool(name="rope", bufs=2))
            apool = ctx.enter_context(tc.tile_pool(name="atile", bufs=4))
            npool = ctx.enter_context(tc.tile_pool(name="norm", bufs=2))
            ofpool = ctx.enter_context(tc.tile_pool(name="of", bufs=2))
            ypool = ctx.enter_context(tc.tile_pool(name="y", bufs=2))
            psum = ctx.enter_context(tc.tile_pool(name="ps", bufs=2, space="PSUM"))
            dpool = ctx.enter_context(tc.tile_pool(name="dscr", bufs=4, space="DRAM"))

            # ---- persistent tiles
            wq_sb = persist.tile([128, 8, 512], F32R, tag="wq")
            nc.sync.dma_start(out=wq_sb[:], in_=wqT.ap())
            wk_sb = persist.tile([128, 8, 512], F32R, tag="wk")
            nc.sync.dma_start(out=wk_sb[:], in_=wkT.ap())
            wv_sb = persist.tile([128, 8, 256], F32R, tag="wv")
            nc.sync.dma_start(out=wv_sb[:], in_=wvT.ap())
            wo_sb = persist.tile([128, 2, 1024], BF, tag="wo")
            nc.sync.dma_start(out=wo_sb[:], in_=woT.ap())
            bqk_sb = persist.tile([1, 8, 128], F32R, tag="bqk")
            nc.sync.dma_start(out=bqk_sb[:], in_=bqk.ap())
            bv_sb = persist.tile([1, 256], F32R, tag="bv")
            nc.sync.dma_start(out=bv_sb[:], in_=bv.ap())
            rA_sb = persist.tile([128, S], BF, tag="ra")
            nc.sync.dma_start(out=rA_sb[:], in_=ropeA.ap())
            rB_sb = persist.tile([128, S], BF, tag="rb")
            nc.sync.dma_start(out=rB_sb[:], in_=ropeB.ap())
            mask_sb = persist.tile([128, 128], BF, tag="mask")
            nc.sync.dma_start(out=mask_sb[:], in_=maskd.ap())
            lam_sb = persist.tile([128, 1], F32, tag="lam")
            nc.sync.dma_start(out=lam_sb[:], in_=lamv.ap())
            ii_sb = persist.tile([128, 64], BF, tag="ii")
            nc.sync.dma_start(out=ii_sb[:], in_=iim.ap())

            qT_sb = persist.tile([128, NH, S], BF, tag="qT")
            kT_sb = persist.tile([128, NH, S], BF, tag="kT")
            v_sb = persist.tile([128, 16, NH, 64], BF, tag="v")

            onesr = persist.tile([1, 512], F32R, tag="onesr")
            nc.sync.dma_start(out=onesr[:], in_=onesd.ap())
            ones128 = persist.tile([128, 2], BF, tag="ones128")
            nc.vector.memset(ones128[:], 1.0)
            epst = persist.tile([128, 1], F32, tag="epst")
            nc.vector.memset(epst[:], EPS)

            swap_src = [32, 0, 96, 64]

            for c in range(NCH):
                cs = c * CH
                # ======== projections for this chunk ========
                xc = xpool.tile([128, 8, CH], F32R)
                nc.sync.dma_start(out=xc[:], in_=xT_r[:, :, cs:cs + CH])

                # q/k feature tiles (f 0..3 = q heads, 4..7 = k heads)
                for f in range(8):
                    ps = psum.tile([128, 512], F32, tag="o")
                    wsb = wq_sb if f < 4 else wk_sb
                    fi = f % 4
                    for kt in range(8):
                        nc.tensor.matmul(
                            ps[:], lhsT=wsb[:, kt, fi * 128:fi * 128 + 128],
                            rhs=xc[:, kt, :], start=(kt == 0), stop=False)
                    nc.tensor.matmul(
                        ps[:], lhsT=bqk_sb[:, f, :], rhs=onesr[:],
                        start=False, stop=True)
                    qc = rpool.tile([128, CH], BF, tag="qc")
                    nc.scalar.copy(qc[:], ps[:])
                    qs = rpool.tile([128, CH], BF, tag="qs")
                    for g in range(4):
                        nc.sync.dma_start(
                            out=qs[g * 32:g * 32 + 32, :],
                            in_=qc[swap_src[g]:swap_src[g] + 32, :])
                    t1 = rpool.tile([128, CH], BF, tag="t1")
                    nc.vector.tensor_mul(t1[:], qc[:], rA_sb[:, cs:cs + CH])
                    t2 = rpool.tile([128, CH], BF, tag="t2")
                    nc.vector.tensor_mul(t2[:], qs[:], rB_sb[:, cs:cs + CH])
                    dst = (qT_sb if f < 4 else kT_sb)[:, fi, cs:cs + CH]
                    nc.vector.tensor_add(dst, t1[:], t2[:])

                # v for the 4 key blocks of this chunk (seq-major)
                for m in range(4):
                    ps = psum.tile([128, 512], F32, tag="o")
                    for kt in range(8):
                        nc.tensor.matmul(
                            ps[:, 0:256], lhsT=xc[:, kt, m * 128:m * 128 + 128],
                            rhs=wv_sb[:, kt, :], start=(kt == 0), stop=False)
                    nc.tensor.matmul(
                        ps[:, 0:256], lhsT=onesr[:, 0:128], rhs=bv_sb[:],
                        start=False, stop=True)
                    nc.vector.tensor_copy(
                        v_sb[:, 4 * c + m, :, :],
                        ps[:, 0:256].rearrange("p (h d) -> p h d", h=4))

                # ======== attention for this chunk ========
                njb = 4 * c + 4
                sums1 = psum.tile([128, 512], F32, tag="sums")
                sums2 = psum.tile([128, 512], F32, tag="sums")
                r1t = npool.tile([128, 512], F32, tag="r1t")
                r2t = npool.tile([128, 512], F32, tag="r2t")
                of_sb = ofpool.tile([128, 2, CH], BF, tag="of")

                for hg in range(2):
                    o_ps = [psum.tile([128, 512], F32, tag="o",
                                      name=f"o_ps_{c}_{hg}_{i}")
                            for i in range(2)]
                    for jb in range(njb):
                        i0 = max(0, (jb - 4 * c) * 128)
                        for hh in range(2):
                            h = 2 * hg + hh
                            sc = psum.tile([128, 2, 512], F32, tag="score")
                            nc.tensor.matmul(
                                sc[:, 0, i0:512],
                                lhsT=kT_sb[0:64, h, jb * JB:jb * JB + JB],
                                rhs=qT_sb[0:64, h, cs + i0:cs + CH],
                                start=True, stop=True)
                            nc.tensor.matmul(
                                sc[:, 1, i0:512],
                                lhsT=kT_sb[64:128, h, jb * JB:jb * JB + JB],
                                rhs=qT_sb[64:128, h, cs + i0:cs + CH],
                                start=True, stop=True)
                            at = apool.tile([128, 2, 512], BF, tag="at")
                            nc.scalar.activation(
                                at[:, :, i0:512], sc[:, :, i0:512], AF.Exp,
                                scale=0.125)
                            if jb >= 4 * c:
                                nc.vector.tensor_mul(
                                    at[:, 0, i0:i0 + 128],
                                    at[:, 0, i0:i0 + 128], mask_sb[:])
                                nc.vector.tensor_mul(
                                    at[:, 1, i0:i0 + 128],
                                    at[:, 1, i0:i0 + 128], mask_sb[:])
                            vsl = v_sb[:, jb, h, :]
                            nc.tensor.matmul(
                                o_ps[hh][0:64, i0:512], lhsT=vsl,
                                rhs=at[:, 0, i0:512], start=(jb == 0),
                                stop=(jb == njb - 1), tile_position=(0, 0))
                            nc.tensor.matmul(
                                o_ps[hh][64:128, i0:512], lhsT=vsl,
                                rhs=at[:, 1, i0:512], start=(jb == 0),
                                stop=(jb == njb - 1), tile_position=(0, 64))
                            nc.tensor.matmul(
                                sums1[32 * h:32 * h + 1, i0:512],
                                lhsT=ones128[:, 0:1], rhs=at[:, 0, i0:512],
                                start=(jb == 0), stop=(jb == njb - 1),
                                tile_position=(0, 32 * h))
                            nc.tensor.matmul(
                                sums2[32 * h:32 * h + 1, i0:512],
                                lhsT=ones128[:, 0:1], rhs=at[:, 1, i0:512],
                                start=(jb == 0), stop=(jb == njb - 1),
                                tile_position=(0, 32 * h))

                    # ---- normalize + combine the two heads of this group
                    p0 = 64 * hg
                    nc.vector.tensor_copy(r1t[p0:p0 + 33, :],
                                          sums1[p0:p0 + 33, :])
                    nc.vector.tensor_copy(r2t[p0:p0 + 33, :],
                                          sums2[p0:p0 + 33, :])
                    nc.scalar.activation(r1t[p0:p0 + 33, :],
                                         r1t[p0:p0 + 33, :], AF.Ln)
                    nc.scalar.activation(r1t[p0:p0 + 33, :],
                                         r1t[p0:p0 + 33, :], AF.Exp,
                                         scale=-1.0)
                    nc.scalar.activation(r2t[p0:p0 + 33, :],
                                         r2t[p0:p0 + 33, :], AF.Ln)
                    nc.scalar.activation(r2t[p0:p0 + 33, :],
                                         r2t[p0:p0 + 33, :], AF.Exp,
                                         scale=-1.0)
                    nc.vector.tensor_scalar(
                        r2t[p0:p0 + 33, :], r2t[p0:p0 + 33, :],
                        lam_sb[p0:p0 + 33, 0:1], None,
                        op0=mybir.AluOpType.mult)

                    occs = []
                    inv_bf = npool.tile([128, 512], BF, tag="invbf")
                    sqs = npool.tile([128, 512], F32, tag="sqs")
                    ssq = psum.tile([128, 512], F32, tag="score")
                    for hh in range(2):
                        h = 2 * hg + hh
                        row = p0 + 32 * hh
                        rb = npool.tile([128, 512], F32, tag="rbb")
                        d1 = dpool.tile([1, 512], F32, tag="d1",
                                        name=f"d1_{c}_{h}")
                        nc.sync.dma_start(out=d1[:], in_=r1t[row:row + 1, :])
                        nc.sync.dma_start(
                            out=rb[0:64, :], in_=_bcast_dram(bass, d1, 64))
                        d2 = dpool.tile([1, 512], F32, tag="d2",
                                        name=f"d2_{c}_{h}")
                        nc.sync.dma_start(out=d2[:], in_=r2t[row:row + 1, :])
                        nc.sync.dma_start(
                            out=rb[64:128, :], in_=_bcast_dram(bass, d2, 64))
                        tstack = npool.tile([128, 512], BF, tag="tstack")
                        nc.vector.tensor_mul(tstack[0:64, :],
                                             o_ps[hh][0:64, :], rb[0:64, :])
                        nc.vector.tensor_mul(tstack[64:128, :],
                                             o_ps[hh][64:128, :],
                                             rb[64:128, :])
                        oc = psum.tile([128, 512], F32, tag="score")
                        nc.tensor.matmul(
                            oc[64 * hh:64 * hh + 64, :], lhsT=ii_sb[:],
                            rhs=tstack[:], start=True, stop=True,
                            tile_position=(0, 64 * hh))
                        occ = npool.tile([128, 512], BF, tag="occ")
                        nc.vector.tensor_copy(occ[64 * hh:64 * hh + 64, :],
                                              oc[64 * hh:64 * hh + 64, :])
                        sq = npool.tile([128, 512], BF, tag="sq")
                        nc.vector.tensor_mul(sq[64 * hh:64 * hh + 64, :],
                                             occ[64 * hh:64 * hh + 64, :],
                                             occ[64 * hh:64 * hh + 64, :])
                        nc.tensor.matmul(
                            ssq[32 * h:32 * h + 1, :],
                            lhsT=ones128[64 * hh:64 * hh + 64, 0:1],
                            rhs=sq[64 * hh:64 * hh + 64, :],
                            start=True, stop=True,
                            tile_position=(64 * hh, 32 * h))
                        occs.append(occ)
                        if hh == 1:
                            nc.vector.tensor_copy(sqs[p0:p0 + 33, :],
                                                  ssq[p0:p0 + 33, :])
                            nc.scalar.activation(sqs[p0:p0 + 33, :],
                                                 sqs[p0:p0 + 33, :], AF.Ln,
                                                 scale=1.0 / 64.0,
                                                 bias=epst[p0:p0 + 33, 0:1])
                            nc.scalar.activation(inv_bf[p0:p0 + 33, :],
                                                 sqs[p0:p0 + 33, :], AF.Exp,
                                                 scale=-0.5)
                    for hh in range(2):
                        h = 2 * hg + hh
                        row = p0 + 32 * hh
                        invb = npool.tile([128, 512], BF, tag="invb")
                        d3 = dpool.tile([1, 512], BF, tag="d3",
                                        name=f"d3_{c}_{h}")
                        nc.sync.dma_start(out=d3[:],
                                          in_=inv_bf[row:row + 1, :])
                        nc.sync.dma_start(
                            out=invb[64 * hh:64 * hh + 64, :],
                            in_=_bcast_dram(bass, d3, 64))
                        occ = occs[hh]
                        nc.vector.tensor_mul(
                            of_sb[64 * hh:64 * hh + 64, hg, :],
                            occ[64 * hh:64 * hh + 64, :],
                            invb[64 * hh:64 * hh + 64, :])

                # ======== partial out-projection for this chunk ========
                for t in range(4):
                    for n in range(2):
                        yp = psum.tile([128, 512], F32, tag="o")
                        for kt in range(2):
                            nc.tensor.matmul(
                                yp[:], lhsT=of_sb[:, kt, t * 128:t * 128 + 128],
                                rhs=wo_sb[:, kt, n * 512:n * 512 + 512],
                                start=(kt == 0), stop=(kt == 1))
                        ys = ypool.tile([128, 512], F32, tag="ys")
                        nc.vector.tensor_copy(ys[:], yp[:])
                        nc.sync.dma_start(
                            out=y_out.ap()[cs + t * 128:cs + t * 128 + 128,
                                           n * 512:n * 512 + 512],
                            in_=ys[:])

    nc.compile()
    return nc


def _bcast_dram(bass_mod, dtile, nparts):
    """AP reading a [1, N] DRAM scratch tile nparts times (row broadcast)."""
    ap = dtile[:]
    return bass_mod.AP(tensor=ap.tensor, offset=ap.offset,
                       ap=[[0, nparts]] + ap.ap[1:])


def _prep_inputs(x, Wq, bq, Wk, bk, Wv, bv, Wo, bo, head_norm_w,
                 lq1, lk1, lq2, lk2):
    lam_full = (LAMBDA_INIT
                + np.exp(np.sum(lq1 * lk1, -1))
                - np.exp(np.sum(lq2 * lk2, -1)))  # [H]

    half = HALF
    inv_freq = 1.0 / (10000.0 ** (np.arange(half, dtype=np.float64) / half))
    ang = np.arange(S, dtype=np.float64)[:, None] * inv_freq[None, :]  # [S,32]
    cosT = np.cos(ang).T.astype(np.float32)  # [32, S]
    sinT = np.sin(ang).T.astype(np.float32)
    ropeA = np.tile(cosT, (4, 1)).astype(BF16)                      # [128,S]
    ropeB = np.concatenate([-sinT, sinT, -sinT, sinT], 0).astype(BF16)

    maskd = np.triu(np.ones((128, 128), np.float32)).astype(BF16)   # j<=i
    iim = np.zeros((128, 64), np.float32)
    iim[np.arange(128), np.arange(128) % 64] = 1.0
    iim = iim.astype(BF16)

    in_maps = []
    for c in range(NCORES):
        b = c // 4
        h0 = 4 * (c % 4)
        rq = slice(h0 * 128, h0 * 128 + 512)
        rv = slice(h0 * 64, h0 * 64 + 256)

        xTc = np.ascontiguousarray(x[b].T)                          # [D, S]
        wq_l = Wq[rq].T  # [1024, 512]
        wk_l = Wk[rq].T
        wv_l = Wv[rv].T  # [1024, 256]
        wqr = np.ascontiguousarray(
            wq_l.reshape(8, 128, 512).transpose(1, 0, 2))
        wkr = np.ascontiguousarray(
            wk_l.reshape(8, 128, 512).transpose(1, 0, 2))
        wvr = np.ascontiguousarray(
            wv_l.reshape(8, 128, 256).transpose(1, 0, 2))

        hnw = head_norm_w[h0:h0 + 4].reshape(256)                   # local dims
        wo_l = Wo[:, rv].T * (hnw * (1.0 - LAMBDA_INIT))[:, None]   # [256,1024]
        wor = np.ascontiguousarray(
            wo_l.reshape(2, 128, 1024).transpose(1, 0, 2)).astype(BF16)

        bqk_arr = np.stack([bq[rq][f * 128:f * 128 + 128] if f < 4
                            else bk[rq][(f - 4) * 128:(f - 4) * 128 + 128]
                            for f in range(8)]).astype(np.float32)  # [8,128]
        bv_arr = bv[rv].reshape(1, 256).astype(np.float32)

        lamv = np.ones((128, 1), np.float32)
        for hl in range(4):
            lamv[32 * hl, 0] = -lam_full[h0 + hl]

        in_maps.append({
            "xT": xTc.astype(np.float32),
            "wqT": wqr.astype(np.float32),
            "wkT": wkr.astype(np.float32),
            "wvT": wvr.astype(np.float32),
            "bqk": bqk_arr,
            "bv": bv_arr,
            "woT": wor,
            "ropeA": ropeA,
            "ropeB": ropeB,
            "maskd": maskd,
            "lamv": lamv,
            "iim": iim,
            "onesd": np.ones((1, 512), np.float32),
        })
    return in_maps


def kernel(**inputs):
    from concourse.bass_utils import run_bass_kernel_spmd

    if "nc" not in _cache:
        _cache["nc"] = _build()
    nc = _cache["nc"]

    inputs = {k: np.asarray(v) for k, v in inputs.items()}
    in_maps = _prep_inputs(**inputs)
    res = run_bass_kernel_spmd(nc, in_maps, list(range(NCORES)))

    bo = inputs["bo"]
    y = np.zeros((B, S, D), np.float32)
    for b in range(B):
        acc = np.zeros((S, D), np.float32)
        for c in range(4 * b, 4 * b + 4):
            acc += res.results[c]["y"]
        y[b] = acc + bo[None, :]
    return y


# revision 10
# speedup vs baseline: 5288.6609x; 5288.6609x over previous
"""Differential self-attention on 8 Trainium2 NeuronCores.

Sharding: batch x head-group. Core c handles batch b = c//4 and heads
hs = 4*(c%4) .. 4*(c%4)+4 (4 of 16 heads). Each core computes q/k/v
projections for its heads, RoPE, both causal softmax score matrices
(flash-style, unnormalized, with matmul-computed row sums), the
differential combination + RMS norm, and a partial out-projection over
its heads' dims. Host sums the 4 partial y per batch and adds bo.

Per-core kernel layouts (feature-major "T" = [feat, seq]):
  qT/kT  [128, 4, 2048]  rows 0:64 = component 1, 64:128 = component 2
  v      [128, 16, 4, 65] (key-block, head, hd + ones col [unused])
  scores sT [j=128, i<=512] so attn@v contracts j on partitions.
"""

import math

import numpy as np
import ml_dtypes

B, S, D = 2, 2048, 1024
H, HD = 16, 64
HALF = HD // 2
NCORES = 8
NH = 4            # heads per core
LAMBDA_INIT = 0.2
EPS = 1e-6
CH = 512          # query chunk
NCH = S // CH
JB = 128          # key block
BF16 = ml_dtypes.bfloat16

_cache = {}


def _build():
    import concourse.bass as bass
    import concourse.tile as tile
    from concourse import bacc, mybir

    F32R = mybir.dt.float32r
    F32 = mybir.dt.float32
    BF = mybir.dt.bfloat16
    AF = mybir.ActivationFunctionType

    nc = bacc.Bacc("TRN2", debug=False, num_devices=NCORES)

    xT = nc.dram_tensor("xT", [D, S], F32R, kind="ExternalInput")
    wqT = nc.dram_tensor("wqT", [128, 8, 512], F32R, kind="ExternalInput")
    wkT = nc.dram_tensor("wkT", [128, 8, 512], F32R, kind="ExternalInput")
    wvT = nc.dram_tensor("wvT", [128, 8, 256], F32R, kind="ExternalInput")
    bqk = nc.dram_tensor("bqk", [8, 128], F32R, kind="ExternalInput")
    bv = nc.dram_tensor("bv", [1, 256], F32R, kind="ExternalInput")
    woT = nc.dram_tensor("woT", [128, 2, 1024], F32R, kind="ExternalInput")
    ropeA = nc.dram_tensor("ropeA", [128, S], BF, kind="ExternalInput")
    ropeB = nc.dram_tensor("ropeB", [128, S], BF, kind="ExternalInput")
    maskd = nc.dram_tensor("maskd", [128, 128], BF, kind="ExternalInput")
    lamv = nc.dram_tensor("lamv", [128, 1], F32, kind="ExternalInput")
    iim = nc.dram_tensor("iim", [128, 64], BF, kind="ExternalInput")
    onesd = nc.dram_tensor("onesd", [1, 512], F32R, kind="ExternalInput")
    ones128d = nc.dram_tensor("ones128d", [128, 2], F32R, kind="ExternalInput")
    y_out = nc.dram_tensor("y", [S, D], F32, kind="ExternalOutput")

    xT_r = xT.ap().rearrange("(t p) s -> p t s", p=128)

    with tile.TileContext(nc) as tc:
        import contextlib
        ctx = contextlib.ExitStack()
        with ctx:
            persist = ctx.enter_context(tc.tile_pool(name="persist", bufs=1))
            xpool = ctx.enter_context(tc.tile_pool(name="xc", bufs=2))
            rpool = ctx.enter_context(tc.tile_pool(name="rope", bufs=2))
            apool = ctx.enter_context(tc.tile_pool(name="atile", bufs=4))
            npool = ctx.enter_context(tc.tile_pool(name="norm", bufs=2))
            ofpool = ctx.enter_context(tc.tile_pool(name="of", bufs=2))
            ypool = ctx.enter_context(tc.tile_pool(name="y", bufs=2))
            psum = ctx.enter_context(tc.tile_pool(name="ps", bufs=2, space="PSUM"))
            dpool = ctx.enter_context(tc.tile_pool(name="dscr", bufs=4, space="DRAM"))

            # ---- persistent tiles
            wq_sb = persist.tile([128, 8, 512], F32R, tag="wq")
            nc.sync.dma_start(out=wq_sb[:], in_=wqT.ap())
            wk_sb = persist.tile([128, 8, 512], F32R, tag="wk")
            nc.sync.dma_start(out=wk_sb[:], in_=wkT.ap())
            wv_sb = persist.tile([128, 8, 256], F32R, tag="wv")
            nc.sync.dma_start(out=wv_sb[:], in_=wvT.ap())
            wo_sb = persist.tile([128, 2, 1024], F32R, tag="wo")
            nc.sync.dma_start(out=wo_sb[:], in_=woT.ap())
            bqk_sb = persist.tile([1, 8, 128], F32R, tag="bqk")
            nc.sync.dma_start(out=bqk_sb[:], in_=bqk.ap())
            bv_sb = persist.tile([1, 256], F32R, tag="bv")
            nc.sync.dma_start(out=bv_sb[:], in_=bv.ap())
            rA_sb = persist.tile([128, S], BF, tag="ra")
            nc.sync.dma_start(out=rA_sb[:], in_=ropeA.ap())
            rB_sb = persist.tile([128, S], BF, tag="rb")
            nc.sync.dma_start(out=rB_sb[:], in_=ropeB.ap())
            mask_sb = persist.tile([128, 128], BF, tag="mask")
            nc.sync.dma_start(out=mask_sb[:], in_=maskd.ap())
            lam_sb = persist.tile([128, 1], F32, tag="lam")
            nc.sync.dma_start(out=lam_sb[:], in_=lamv.ap())
            ii_sb = persist.tile([128, 64], BF, tag="ii")
            nc.sync.dma_start(out=ii_sb[:], in_=iim.ap())

            qT_sb = persist.tile([128, NH, S], BF, tag="qT")
            kT_sb = persist.tile([128, NH, S], BF, tag="kT")
            v_sb = persist.tile([128, 16, NH, 64], BF, tag="v")

            onesr = persist.tile([1, 512], F32R, tag="onesr")
            nc.sync.dma_start(out=onesr[:], in_=onesd.ap())
            ones128 = persist.tile([128, 2], BF, tag="ones128")
            nc.vector.memset(ones128[:], 1.0)
            ones128r = persist.tile([128, 2], F32R, tag="ones128r")
            nc.sync.dma_start(out=ones128r[:], in_=ones128d.ap())
            epst = persist.tile([128, 1], F32, tag="epst")
            nc.vector.memset(epst[:], EPS)

            swap_src = [32, 0, 96, 64]

            for c in range(NCH):
                cs = c * CH
                # ======== projections for this chunk ========
                xc = xpool.tile([128, 8, CH], F32R)
                nc.sync.dma_start(out=xc[:], in_=xT_r[:, :, cs:cs + CH])

                # q/k feature tiles (f 0..3 = q heads, 4..7 = k heads)
                for f in range(8):
                    ps = psum.tile([128, 512], F32, tag="o")
                    wsb = wq_sb if f < 4 else wk_sb
                    fi = f % 4
                    for kt in range(8):
                        nc.tensor.matmul(
                            ps[:], lhsT=wsb[:, kt, fi * 128:fi * 128 + 128],
                            rhs=xc[:, kt, :], start=(kt == 0), stop=False)
                    nc.tensor.matmul(
                        ps[:], lhsT=bqk_sb[:, f, :], rhs=onesr[:],
                        start=False, stop=True)
                    qc = rpool.tile([128, CH], BF, tag="qc")
                    nc.scalar.copy(qc[:], ps[:])
                    qs = rpool.tile([128, CH], BF, tag="qs")
                    for g in range(4):
                        nc.sync.dma_start(
                            out=qs[g * 32:g * 32 + 32, :],
                            in_=qc[swap_src[g]:swap_src[g] + 32, :])
                    t1 = rpool.tile([128, CH], BF, tag="t1")
                    nc.vector.tensor_mul(t1[:], qc[:], rA_sb[:, cs:cs + CH])
                    t2 = rpool.tile([128, CH], BF, tag="t2")
                    nc.vector.tensor_mul(t2[:], qs[:], rB_sb[:, cs:cs + CH])
                    dst = (qT_sb if f < 4 else kT_sb)[:, fi, cs:cs + CH]
                    nc.vector.tensor_add(dst, t1[:], t2[:])

                # v for the 4 key blocks of this chunk (seq-major)
                for m in range(4):
                    ps = psum.tile([128, 512], F32, tag="o")
                    for kt in range(8):
                        nc.tensor.matmul(
                            ps[:, 0:256], lhsT=xc[:, kt, m * 128:m * 128 + 128],
                            rhs=wv_sb[:, kt, :], start=(kt == 0), stop=False)
                    nc.tensor.matmul(
                        ps[:, 0:256], lhsT=onesr[:, 0:128], rhs=bv_sb[:],
                        start=False, stop=True)
                    nc.vector.tensor_copy(
                        v_sb[:, 4 * c + m, :, :],
                        ps[:, 0:256].rearrange("p (h d) -> p h d", h=4))

                # ======== attention for this chunk ========
                njb = 4 * c + 4
                sums1 = psum.tile([128, 512], F32, tag="sums")
                sums2 = psum.tile([128, 512], F32, tag="sums")
                r1t = npool.tile([128, 512], F32, tag="r1t")
                r2t = npool.tile([128, 512], F32, tag="r2t")
                of_sb = ofpool.tile([128, 2, CH], F32R, tag="of")

                for hg in range(2):
                    o_ps = [psum.tile([128, 512], F32, tag="o",
                                      name=f"o_ps_{c}_{hg}_{i}")
                            for i in range(2)]
                    for jb in range(njb):
                        i0 = max(0, (jb - 4 * c) * 128)
                        for hh in range(2):
                            h = 2 * hg + hh
                            sc = psum.tile([128, 2, 512], F32, tag="score")
                            nc.tensor.matmul(
                                sc[:, 0, i0:512],
                                lhsT=kT_sb[0:64, h, jb * JB:jb * JB + JB],
                                rhs=qT_sb[0:64, h, cs + i0:cs + CH],
                                start=True, stop=True)
                            nc.tensor.matmul(
                                sc[:, 1, i0:512],
                                lhsT=kT_sb[64:128, h, jb * JB:jb * JB + JB],
                                rhs=qT_sb[64:128, h, cs + i0:cs + CH],
                                start=True, stop=True)
                            at = apool.tile([128, 2, 512], BF, tag="at")
                            nc.scalar.activation(
                                at[:, :, i0:512], sc[:, :, i0:512], AF.Exp,
                                scale=0.125)
                            if jb >= 4 * c:
                                nc.vector.tensor_mul(
                                    at[:, 0, i0:i0 + 128],
                                    at[:, 0, i0:i0 + 128], mask_sb[:])
                                nc.vector.tensor_mul(
                                    at[:, 1, i0:i0 + 128],
                                    at[:, 1, i0:i0 + 128], mask_sb[:])
                            vsl = v_sb[:, jb, h, :]
                            nc.tensor.matmul(
                                o_ps[hh][0:64, i0:512], lhsT=vsl,
                                rhs=at[:, 0, i0:512], start=(jb == 0),
                                stop=(jb == njb - 1), tile_position=(0, 0))
                            nc.tensor.matmul(
                                o_ps[hh][64:128, i0:512], lhsT=vsl,
                                rhs=at[:, 1, i0:512], start=(jb == 0),
                                stop=(jb == njb - 1), tile_position=(0, 64))
                            nc.tensor.matmul(
                                sums1[32 * h:32 * h + 1, i0:512],
                                lhsT=ones128[:, 0:1], rhs=at[:, 0, i0:512],
                                start=(jb == 0), stop=(jb == njb - 1),
                                tile_position=(0, 32 * h))
                            nc.tensor.matmul(
                                sums2[32 * h:32 * h + 1, i0:512],
                                lhsT=ones128[:, 0:1], rhs=at[:, 1, i0:512],
                                start=(jb == 0), stop=(jb == njb - 1),
                                tile_position=(0, 32 * h))

                    # ---- normalize + combine the two heads of this group
                    p0 = 64 * hg
                    nc.vector.tensor_copy(r1t[p0:p0 + 33, :],
                                          sums1[p0:p0 + 33, :])
                    nc.vector.tensor_copy(r2t[p0:p0 + 33, :],
                                          sums2[p0:p0 + 33, :])
                    nc.scalar.activation(r1t[p0:p0 + 33, :],
                                         r1t[p0:p0 + 33, :], AF.Ln)
                    nc.scalar.activation(r1t[p0:p0 + 33, :],
                                         r1t[p0:p0 + 33, :], AF.Exp,
                                         scale=-1.0)
                    nc.scalar.activation(r2t[p0:p0 + 33, :],
                                         r2t[p0:p0 + 33, :], AF.Ln)
                    nc.scalar.activation(r2t[p0:p0 + 33, :],
                                         r2t[p0:p0 + 33, :], AF.Exp,
                                         scale=-1.0)
                    nc.vector.tensor_scalar(
                        r2t[p0:p0 + 33, :], r2t[p0:p0 + 33, :],
                        lam_sb[p0:p0 + 33, 0:1], None,
                        op0=mybir.AluOpType.mult)

                    occs = []
                    inv_bf = npool.tile([128, 512], F32, tag="invbf")
                    sqs = npool.tile([128, 512], F32, tag="sqs")
                    ssq = psum.tile([128, 512], F32, tag="score")
                    for hh in range(2):
                        h = 2 * hg + hh
                        row = p0 + 32 * hh
                        rb = npool.tile([128, 512], F32, tag="rbb")
                        d1 = dpool.tile([1, 512], F32, tag="d1",
                                        name=f"d1_{c}_{h}")
                        nc.sync.dma_start(out=d1[:], in_=r1t[row:row + 1, :])
                        nc.sync.dma_start(
                            out=rb[0:64, :], in_=_bcast_dram(bass, d1, 64))
                        d2 = dpool.tile([1, 512], F32, tag="d2",
                                        name=f"d2_{c}_{h}")
                        nc.sync.dma_start(out=d2[:], in_=r2t[row:row + 1, :])
                        nc.sync.dma_start(
                            out=rb[64:128, :], in_=_bcast_dram(bass, d2, 64))
                        tstack = npool.tile([128, 512], BF, tag="tstack")
                        nc.vector.tensor_mul(tstack[0:64, :],
                                             o_ps[hh][0:64, :], rb[0:64, :])
                        nc.vector.tensor_mul(tstack[64:128, :],
                                             o_ps[hh][64:128, :],
                                             rb[64:128, :])
                        oc = psum.tile([128, 512], F32, tag="score")
                        nc.tensor.matmul(
                            oc[64 * hh:64 * hh + 64, :], lhsT=ii_sb[:],
                            rhs=tstack[:], start=True, stop=True,
                            tile_position=(0, 64 * hh))
                        occ = npool.tile([128, 512], F32, tag="occ")
                        nc.vector.tensor_copy(occ[64 * hh:64 * hh + 64, :],
                                              oc[64 * hh:64 * hh + 64, :])
                        sq = npool.tile([128, 512], BF, tag="sq")
                        nc.vector.tensor_mul(sq[64 * hh:64 * hh + 64, :],
                                             occ[64 * hh:64 * hh + 64, :],
                                             occ[64 * hh:64 * hh + 64, :])
                        nc.tensor.matmul(
                            ssq[32 * h:32 * h + 1, :],
                            lhsT=ones128[64 * hh:64 * hh + 64, 0:1],
                            rhs=sq[64 * hh:64 * hh + 64, :],
                            start=True, stop=True,
                            tile_position=(64 * hh, 32 * h))
                        occs.append(occ)
                        if hh == 1:
                            nc.vector.tensor_copy(sqs[p0:p0 + 33, :],
                                                  ssq[p0:p0 + 33, :])
                            nc.scalar.activation(sqs[p0:p0 + 33, :],
                                                 sqs[p0:p0 + 33, :], AF.Ln,
                                                 scale=1.0 / 64.0,
                                                 bias=epst[p0:p0 + 33, 0:1])
                            nc.scalar.activation(inv_bf[p0:p0 + 33, :],
                                                 sqs[p0:p0 + 33, :], AF.Exp,
                                                 scale=-0.5)
                    for hh in range(2):
                        h = 2 * hg + hh
                        row = p0 + 32 * hh
                        invb = npool.tile([128, 512], F32, tag="invb")
                        d3 = dpool.tile([1, 512], F32, tag="d3",
                                        name=f"d3_{c}_{h}")
                        nc.sync.dma_start(out=d3[:],
                                          in_=inv_bf[row:row + 1, :])
                        nc.sync.dma_start(
                            out=invb[64 * hh:64 * hh + 64, :],
                            in_=_bcast_dram(bass, d3, 64))
                        occ = occs[hh]
                        nc.vector.tensor_mul(
                            of_sb[64 * hh:64 * hh + 64, hg, :],
                            occ[64 * hh:64 * hh + 64, :],
                            invb[64 * hh:64 * hh + 64, :])

                # ======== partial out-projection for this chunk ========
                for t in range(4):
                    for n in range(2):
                        yp = psum.tile([128, 512], F32, tag="o")
                        for kt in range(2):
                            nc.tensor.matmul(
                                yp[:], lhsT=of_sb[:, kt, t * 128:t * 128 + 128],
                                rhs=wo_sb[:, kt, n * 512:n * 512 + 512],
                                start=(kt == 0), stop=(kt == 1))
                        ys = ypool.tile([128, 512], F32, tag="ys")
                        nc.vector.tensor_copy(ys[:], yp[:])
                        nc.sync.dma_start(
                            out=y_out.ap()[cs + t * 128:cs + t * 128 + 128,
                                           n * 512:n * 512 + 512],
                            in_=ys[:])

    nc.compile()
    return nc


def _bcast_dram(bass_mod, dtile, nparts):
    """AP reading a [1, N] DRAM scratch tile nparts times (row broadcast)."""
    ap = dtile[:]
    return bass_mod.AP(tensor=ap.tensor, offset=ap.offset,
                       ap=[[0, nparts]] + ap.ap[1:])


def _prep_inputs(x, Wq, bq, Wk, bk, Wv, bv, Wo, bo, head_norm_w,
                 lq1, lk1, lq2, lk2):
    lam_full = (LAMBDA_INIT
                + np.exp(np.sum(lq1 * lk1, -1))
                - np.exp(np.sum(lq2 * lk2, -1)))  # [H]

    half = HALF
    inv_freq = 1.0 / (10000.0 ** (np.arange(half, dtype=np.float64) / half))
    ang = np.arange(S, dtype=np.float64)[:, None] * inv_freq[None, :]  # [S,32]
    cosT = np.cos(ang).T.astype(np.float32)  # [32, S]
    sinT = np.sin(ang).T.astype(np.float32)
    ropeA = np.tile(cosT, (4, 1)).astype(BF16)                      # [128,S]
    ropeB = np.concatenate([-sinT, sinT, -sinT, sinT], 0).astype(BF16)

    maskd = np.triu(np.ones((128, 128), np.float32)).astype(BF16)   # j<=i
    iim = np.zeros((128, 64), np.float32)
    iim[np.arange(128), np.arange(128) % 64] = 1.0
    iim = iim.astype(BF16)

    in_maps = []
    for c in range(NCORES):
        b = c // 4
        h0 = 4 * (c % 4)
        rq = slice(h0 * 128, h0 * 128 + 512)
        rv = slice(h0 * 64, h0 * 64 + 256)

        xTc = np.ascontiguousarray(x[b].T)                          # [D, S]
        wq_l = Wq[rq].T  # [1024, 512]
        wk_l = Wk[rq].T
        wv_l = Wv[rv].T  # [1024, 256]
        wqr = np.ascontiguousarray(
            wq_l.reshape(8, 128, 512).transpose(1, 0, 2))
        wkr = np.ascontiguousarray(
            wk_l.reshape(8, 128, 512).transpose(1, 0, 2))
        wvr = np.ascontiguousarray(
            wv_l.reshape(8, 128, 256).transpose(1, 0, 2))

        hnw = head_norm_w[h0:h0 + 4].reshape(256)                   # local dims
        wo_l = Wo[:, rv].T * (hnw * (1.0 - LAMBDA_INIT))[:, None]   # [256,1024]
        wor = np.ascontiguousarray(
            wo_l.reshape(2, 128, 1024).transpose(1, 0, 2)).astype(np.float32)

        bqk_arr = np.stack([bq[rq][f * 128:f * 128 + 128] if f < 4
                            else bk[rq][(f - 4) * 128:(f - 4) * 128 + 128]
                            for f in range(8)]).astype(np.float32)  # [8,128]
        bv_arr = bv[rv].reshape(1, 256).astype(np.float32)

        lamv = np.ones((128, 1), np.float32)
        for hl in range(4):
            lamv[32 * hl, 0] = -lam_full[h0 + hl]

        in_maps.append({
            "xT": xTc.astype(np.float32),
            "wqT": wqr.astype(np.float32),
            "wkT": wkr.astype(np.float32),
            "wvT": wvr.astype(np.float32),
            "bqk": bqk_arr,
            "bv": bv_arr,
            "woT": wor,
            "ropeA": ropeA,
            "ropeB": ropeB,
            "maskd": maskd,
            "lamv": lamv,
            "iim": iim,
            "onesd": np.ones((1, 512), np.float32),
            "ones128d": np.ones((128, 2), np.float32),
        })
    return in_maps


def kernel(**inputs):
    from concourse.bass_utils import run_bass_kernel_spmd

    if "nc" not in _cache:
        _cache["nc"] = _build()
    nc = _cache["nc"]

    inputs = {k: np.asarray(v) for k, v in inputs.items()}
    in_maps = _prep_inputs(**inputs)
    res = run_bass_kernel_spmd(nc, in_maps, list(range(NCORES)))

    bo = inputs["bo"]
    y = np.zeros((B, S, D), np.float32)
    for b in range(B):
        acc = np.zeros((S, D), np.float32)
        for c in range(4 * b, 4 * b + 4):
            acc += res.results[c]["y"]
        y[b] = acc + bo[None, :]
    return y
